# revision 23
# baseline (speedup 1.0000x reference)
"""Trainium2 Bass kernel for nn_BiAlignLayer.

Reference computation:
    weight   = einsum('bld,bmd->blm', i, j)
    weight_i = softmax(weight, axis=-1)   # rows sum to 1 over m
    weight_j = softmax(weight, axis=1)    # cols sum to 1 over l
    weighted_i = einsum('blm,bld->bmd', weight_i, i)
    weighted_j = einsum('blm,bmd->bld', weight_j, j)
    oi = relu(mean_l(i - weighted_j) @ W + b)
    oj = relu(mean_m(j - weighted_i) @ W + b)
    out = 0.5 * (oi + oj)

Because mean_m(weighted_i) = mean_l(i) (softmax over m sums to 1) and
mean_l(weighted_j) = mean_m(j) (softmax over l sums to 1), the whole
attention block drops out of the final means:
    u   = mean_l(i) - mean_l(j)                       # [B, D]
    out = 0.5 * (relu(u @ W + b) + relu(-(u @ W) + b))
so with t = (u@W + b)/2 and m = (b - u@W)/2 the output is
    out = relu(t) + relu(m).
The kernel computes exactly that shape of work; the only approximation is
that i, j and W are down-converted to float16 while being DMA'd into SBUF
(casting DMAs ride the Pool/SWDGE path). All accumulation stays in fp32
PSUM, so the end-to-end relative error is ~3e-4 against the fp32
reference -- far inside the 2e-2 gate -- while the HBM->SBUF stream
(the roofline term for this kernel) halves versus fp32.

Per-core structure:
  * One casting Pool DMA per (tensor, batch element) streams i and j as
    [128, 8x512] f16 tiles (1 MB of SBUF writes each) back-to-back on
    the DMA engines; consecutive DRAM rows pack per partition line so
    each DMA is 128 fat descriptors.  A small f32 HWDGE "bridge" DMA
    covers the first 128 rows during the window where the first SWDGE
    descriptor generation is still running.
  * The L-reduction runs "transposed" on the tensor engine: each
    [128, 128] data chunk is the matmul *stationary* and a tiny signed
    one-hot selector column (+-1/(2L), exact in f16) is the moving
    operand, accumulating uT[d, b] directly in PSUM.  Each of the 4
    d-chunk accumulation chains owns a full PSUM bank (2 KB stride) so
    the four chains stay open concurrently across the whole stream.
    This also removes the u transpose from the tail entirely.
  * W is cast to f16 by one more Pool DMA queued after the data stream
    (the uT copies below hide inside its transfer + sem window); b stays
    fp32 via a tiny HWDGE DMA.
  * Tail: +uT and -uT are copy-cast side by side into one [128, 8]-wide
    f16 moving tile, so t and m come from a single 5-matmul chain per
    n-chunk (shared W stationary, rank-1 bias with a 0.5-valued rhs) and
    land interleaved in one PSUM bank; the epilogue is one whole-tile
    relu and one strided add on the DVE, then a single output DMA.

Sharding: data-parallel over batch, 4 batch elements per core x 8 cores.
"""

import sys

import numpy as np

if "/opt/trn_rl_repo" not in sys.path:
    sys.path.insert(0, "/opt/trn_rl_repo")

import concourse.mybir as mybir
import concourse.tile as tile
from concourse import bacc
from concourse.bass import ds
from concourse.bass_utils import run_bass_kernel_spmd

B = 32            # total batch
NCORES = 8
NB = B // NCORES  # batches per core
L = 1024
D = 512
NN = 512          # output feature dim (2 * nn_dim)
P = 128
DCH = D // P      # 128-col d-chunks
NCH = NN // P     # 128-row n-chunks
F32 = mybir.dt.float32
F16 = mybir.dt.float16
PSB = 512         # one PSUM bank, in fp32 elements per partition

_CACHE = {}


def _build_bass(reps=1):
    """Build the per-core Bass program. reps>1 repeats the body (for the
    wall-clock marginal benchmark); outputs are simply overwritten."""
    nc = bacc.Bacc("TRN2", debug=False)

    i_dram = nc.declare_dram_parameter("i", [NB * L, D], F32, isOutput=False)
    j_dram = nc.declare_dram_parameter("j", [NB * L, D], F32, isOutput=False)
    w_dram = nc.declare_dram_parameter("w", [D, NN], F32, isOutput=False)
    b_dram = nc.declare_dram_parameter("b", [1, NN], F32, isOutput=False)
    o_dram = nc.declare_dram_parameter("out", [NN, NB], F32, isOutput=True)

    # out[cn*P + p, b] <- o_sb[p, cn*NB + b]
    o_view = o_dram.ap().rearrange("(c p) b -> p c b", p=P)

    with tile.TileContext(nc) as tc:
        with (
            tc.tile_pool(name="consts", bufs=1) as consts,
            tc.tile_pool(name="data", bufs=6) as data,
            tc.tile_pool(name="small", bufs=1) as small,
            tc.tile_pool(name="psum", bufs=1, space="PSUM") as psum,
        ):
            # Signed one-hot selector columns, pre-scaled by 1/(2L) (an
            # exact power of two in f16): block b has column b = +1/(2L)
            # for i tiles, block NB+b has column b = -1/(2L) for j tiles.
            # sel32 is the f32 twin of block (i, batch 0) for the f32
            # bridge tile that rides HWDGE while the SWDGE generator spins
            # up.
            s = 1.0 / (2.0 * L)
            sel = consts.tile([P, 2 * NB * NB], F16)
            nc.vector.memset(sel[:], 0.0)
            for b in range(NB):
                nc.vector.memset(sel[:, ds(b * NB + b, 1)], s)
                nc.vector.memset(sel[:, ds((NB + b) * NB + b, 1)], -s)
            sel32 = consts.tile([P, NB], F32)
            nc.vector.memset(sel32[:], 0.0)
            nc.vector.memset(sel32[:, ds(0, 1)], s)

            halfones = consts.tile([1, 2 * NB], F32)
            nc.vector.memset(halfones[:], 0.5)

            w_sb = consts.tile([P, DCH * NN], F16)
            b_sb = consts.tile([1, NN], F32)

            for rep in range(reps):
                _emit_body(
                    nc, data, small, psum,
                    i_dram.ap(), j_dram.ap(), w_dram.ap(), b_dram.ap(),
                    o_view, sel, sel32, halfones, w_sb, b_sb,
                    load_wb=(rep == 0),
                )

    nc.compile()
    return nc


def _emit_body(nc, data, small, psum, i_ap, j_ap, w_ap, b_ap, o_view,
               sel, sel32, halfones, w_sb, b_sb, load_wb=True):
    # PSUM layout (fp32 cols per partition): uT's four d-chunk accumulation
    # chains at bank stride (cols cd*512), then one bank holding the dense
    # t = (v+b)/2 and m = (b-v)/2 results interleaved per n-chunk.
    ut_psum = psum.tile([P, DCH * PSB], F32)
    tm_psum = psum.tile([P, PSB], F32)

    # Bridge tile: the first 128 rows of batch 0's i ride a plain f32
    # HWDGE DMA.  The transfer occupies the DMA engines only during the
    # window where the first SWDGE (Pool) descriptor generation is still
    # running, so it comes to us almost free and shortens the casting
    # stream by one row-group.
    t32 = data.tile([P, D], F32, tag="t32")
    nc.sync.dma_start(out=t32[:], in_=i_ap[ds(0, P), :])

    if load_wb:
        # b is tiny and only feeds the rank-1 bias matmuls of the tail.
        nc.sync.dma_start(out=b_sb[:], in_=b_ap[:])

    # --- phase 1: uT[d, b] = (sum_l i[b,:,d] - sum_l j[b,:,d]) / 2L --------
    # Casting Pool DMAs halve the HBM->SBUF stream (the roofline term);
    # each [128, 128] f16 data chunk is then consumed as a matmul
    # *stationary* with the tiny selector column moving, so the whole
    # reduction costs the tensor engine almost nothing and produces uT in
    # the layout the dense tail wants.  Tiles pack consecutive DRAM rows
    # per partition line, so a whole batch element is one 128-descriptor
    # DMA.  The reduction only needs every row summed once -- which rows a
    # partition holds is irrelevant -- so the row->partition mapping is
    # free to chase descriptor shape.
    #
    # The bridge covers (i, batch 0, rows 0..127): open the four cd chains
    # with its f32 matmuls.
    for cd in range(DCH):
        nc.tensor.matmul(
            ut_psum[:, ds(cd * PSB, NB)],
            t32[:, ds(cd * P, P)],
            sel32[:],
            start=True,
            stop=False,
        )
    n_tiles = 2 * NB
    t_idx = 0
    for b in range(NB):
        for x_ap, blk in ((i_ap, b), (j_ap, NB + b)):
            if t_idx == 0:
                row0, rows = b * L + P, L - P  # bridge took rows 0..127
            else:
                row0, rows = b * L, L
            rp = rows // P  # consecutive rows per partition line
            th = data.tile([P, rp * D], F16, tag="t")
            nc.gpsimd.dma_start(
                out=th[:].rearrange("p (r n) -> p r n", r=rp),
                in_=x_ap[ds(row0, rows), :].rearrange(
                    "(p r) n -> p r n", r=rp
                ),
            )
            for r in range(rp):
                for cd in range(DCH):
                    nc.tensor.matmul(
                        ut_psum[:, ds(cd * PSB, NB)],
                        th[:, ds(r * D + cd * P, P)],
                        sel[:, ds(blk * NB, NB)],
                        start=False,
                        stop=(t_idx == n_tiles - 1 and r == rp - 1),
                    )
            t_idx += 1

    # W is only consumed by the dense tail, so its (casting) DMA queues
    # after the data stream: the last data tile lands ~1.5us earlier and W
    # streams in while the uT copy below runs.
    if load_wb:
        nc.gpsimd.dma_start(
            out=w_sb[:].rearrange("p (c n) -> p c n", c=DCH),
            in_=w_ap.rearrange("(c p) n -> p c n", p=P),
        )

    # --- phase 2: t[n,b] = (v+b)/2, m[n,b] = (b-v)/2, v = sum_d W[d,n] u[b,d]
    # +uT and -uT sit side by side in one [128, 8]-moving tile, so t and m
    # come from a single matmul chain per n-chunk (the stationary W block
    # is shared) and land interleaved in one PSUM bank.  The copies run
    # right after the last data tile and hide inside the W DMA + sem
    # window; the bias enters each chain as a rank-1 (K=1) matmul with a
    # 0.5-valued rhs.
    ut_view = ut_psum[:].rearrange("p (c x) -> p c x", x=PSB)[:, :, ds(0, NB)]
    ut_pm = small.tile([P, DCH * 2 * NB], F16)
    pm_view = ut_pm[:].rearrange("p (c s x) -> p c s x", s=2, x=NB)
    nc.vector.tensor_copy(pm_view[:, :, 0, :], ut_view)
    nc.vector.tensor_scalar_mul(pm_view[:, :, 1, :], ut_view, -1.0)
    for cn in range(NCH):
        for cd in range(DCH):
            nc.tensor.matmul(
                tm_psum[:, ds(cn * 2 * NB, 2 * NB)],
                w_sb[:, ds(cd * NN + cn * P, P)],
                ut_pm[:, ds(cd * 2 * NB, 2 * NB)],
                start=(cd == 0),
                stop=False,
            )
        nc.tensor.matmul(
            tm_psum[:, ds(cn * 2 * NB, 2 * NB)],
            b_sb[:, ds(cn * P, P)],
            halfones[:],
            start=False,
            stop=True,
        )

    # --- phase 3: out = relu(t) + relu(m) ----------------------------------
    r_tm = small.tile([P, NCH * 2 * NB], F32)
    nc.vector.tensor_scalar_max(r_tm[:], tm_psum[:, ds(0, NCH * 2 * NB)], 0.0)
    r_view = r_tm[:].rearrange("p (c s x) -> p c s x", s=2, x=NB)
    o_sb = small.tile([P, NCH * NB], F32)
    nc.vector.tensor_add(
        o_sb[:].rearrange("p (c x) -> p c x", x=NB),
        r_view[:, :, 0, :],
        r_view[:, :, 1, :],
    )
    nc.sync.dma_start(out=o_view, in_=o_sb[:])


def _get_bass():
    if "nc" not in _CACHE:
        _CACHE["nc"] = _build_bass()
    return _CACHE["nc"]


def _make_in_maps(inputs):
    i = np.ascontiguousarray(np.asarray(inputs["i"], dtype=np.float32))
    j = np.ascontiguousarray(np.asarray(inputs["j"], dtype=np.float32))
    w = np.ascontiguousarray(np.asarray(inputs["W_agg"], dtype=np.float32))
    b = np.ascontiguousarray(
        np.asarray(inputs["b_agg"], dtype=np.float32).reshape(1, NN)
    )
    in_maps = []
    for c in range(NCORES):
        in_maps.append(
            {
                "i": i[c * NB : (c + 1) * NB].reshape(NB * L, D),
                "j": j[c * NB : (c + 1) * NB].reshape(NB * L, D),
                "w": w,
                "b": b,
            }
        )
    return in_maps


def run_traced(trace=False, **inputs):
    nc = _get_bass()
    in_maps = _make_in_maps(inputs)
    res = run_bass_kernel_spmd(nc, in_maps, list(range(NCORES)), trace=trace)
    out = np.concatenate(
        [res.results[c]["out"].T for c in range(NCORES)], axis=0
    ).astype(np.float32)
    return out, res


def kernel(**inputs):
    out, _ = run_traced(trace=False, **inputs)
    return out


# revision 30
# speedup vs baseline: 1.0059x; 1.0059x over previous
"""Trainium2 Bass kernel for nn_BiAlignLayer.

Reference computation:
    weight   = einsum('bld,bmd->blm', i, j)
    weight_i = softmax(weight, axis=-1)   # rows sum to 1 over m
    weight_j = softmax(weight, axis=1)    # cols sum to 1 over l
    weighted_i = einsum('blm,bld->bmd', weight_i, i)
    weighted_j = einsum('blm,bmd->bld', weight_j, j)
    oi = relu(mean_l(i - weighted_j) @ W + b)
    oj = relu(mean_m(j - weighted_i) @ W + b)
    out = 0.5 * (oi + oj)

Because mean_m(weighted_i) = mean_l(i) (softmax over m sums to 1) and
mean_l(weighted_j) = mean_m(j) (softmax over l sums to 1), the whole
attention block drops out of the final means:
    u   = mean_l(i) - mean_l(j)                       # [B, D]
    out = 0.5 * (relu(u @ W + b) + relu(-(u @ W) + b))
so with t = (u@W + b)/2 and m = (b - u@W)/2 the output is
    out = relu(t) + relu(m).
The kernel computes exactly that shape of work; the only approximation is
that i, j and W are down-converted to float16 while being DMA'd into SBUF
(casting DMAs ride the Pool/SWDGE path). All accumulation stays in fp32
PSUM, so the end-to-end relative error is ~3e-4 against the fp32
reference -- far inside the 2e-2 gate -- while the HBM->SBUF stream
(the roofline term for this kernel) halves versus fp32.

Per-core structure:
  * One casting Pool DMA per (tensor, batch element) streams i and j as
    [128, 8x512] f16 tiles (1 MB of SBUF writes each) back-to-back on
    the DMA engines; consecutive DRAM rows pack per partition line so
    each DMA is 128 fat descriptors.  A small f32 HWDGE "bridge" DMA
    covers the first 128 rows during the window where the first SWDGE
    descriptor generation is still running.
  * The L-reduction runs "transposed" on the tensor engine: each
    [128, 128] data chunk is the matmul *stationary* and a tiny signed
    one-hot selector column (+-1/(2L), exact in f16) is the moving
    operand, accumulating uT[d, b] directly in PSUM.  Each of the 4
    d-chunk accumulation chains owns a full PSUM bank (2 KB stride) so
    the four chains stay open concurrently across the whole stream.
    This also removes the u transpose from the tail entirely.
  * W is cast to f16 by one more Pool DMA queued after the data stream
    (the uT copies below hide inside its transfer + sem window); b stays
    fp32 via a tiny HWDGE DMA.
  * Tail: +uT and -uT are copy-cast side by side into one [128, 8]-wide
    f16 moving tile, so t and m come from a single 5-matmul chain per
    n-chunk (shared W stationary, rank-1 bias with a 0.5-valued rhs) and
    land interleaved in one PSUM bank; the epilogue is one whole-tile
    relu and one strided add on the DVE, then a single output DMA.

Sharding: data-parallel over batch, 4 batch elements per core x 8 cores.
"""

import sys

import numpy as np

if "/opt/trn_rl_repo" not in sys.path:
    sys.path.insert(0, "/opt/trn_rl_repo")

import concourse.mybir as mybir
import concourse.tile as tile
from concourse import bacc
from concourse.bass import ds
from concourse.bass_utils import run_bass_kernel_spmd

B = 32            # total batch
NCORES = 8
NB = B // NCORES  # batches per core
L = 1024
D = 512
NN = 512          # output feature dim (2 * nn_dim)
P = 128
DCH = D // P      # 128-col d-chunks
NCH = NN // P     # 128-row n-chunks
F32 = mybir.dt.float32
F16 = mybir.dt.float16
PSB = 512         # one PSUM bank, in fp32 elements per partition

_CACHE = {}


def _build_bass(reps=1):
    """Build the per-core Bass program. reps>1 repeats the body (for the
    wall-clock marginal benchmark); outputs are simply overwritten."""
    nc = bacc.Bacc("TRN2", debug=False)

    i_dram = nc.declare_dram_parameter("i", [NB * L, D], F32, isOutput=False)
    j_dram = nc.declare_dram_parameter("j", [NB * L, D], F32, isOutput=False)
    w_dram = nc.declare_dram_parameter("w", [D, NN], F32, isOutput=False)
    b_dram = nc.declare_dram_parameter("b", [1, NN], F32, isOutput=False)
    # The output DRAM buffer keeps o_sb's native [128, cn*NB+b] layout:
    # contiguous 64 B rows make the store 128 fat descriptors (56 ns at the
    # 7 ns/desc floor) instead of 512 thin ones; the host gather transposes.
    o_dram = nc.declare_dram_parameter("out", [P, NCH * NB], F32, isOutput=True)
    o_view = o_dram.ap()

    with tile.TileContext(nc) as tc:
        with (
            tc.tile_pool(name="consts", bufs=1) as consts,
            tc.tile_pool(name="data", bufs=6) as data,
            tc.tile_pool(name="small", bufs=1) as small,
            tc.tile_pool(name="psum", bufs=1, space="PSUM") as psum,
        ):
            # Signed one-hot selector columns, pre-scaled by 1/(2L) (an
            # exact power of two in f16): block b has column b = +1/(2L)
            # for i tiles, block NB+b has column b = -1/(2L) for j tiles.
            # sel32 is the f32 twin of block (i, batch 0) for the f32
            # bridge tile that rides HWDGE while the SWDGE generator spins
            # up.
            s = 1.0 / (2.0 * L)
            sel = consts.tile([P, 2 * NB * NB], F16)
            nc.vector.memset(sel[:], 0.0)
            for b in range(NB):
                nc.vector.memset(sel[:, ds(b * NB + b, 1)], s)
                nc.vector.memset(sel[:, ds((NB + b) * NB + b, 1)], -s)
            sel32 = consts.tile([P, NB], F32)
            nc.vector.memset(sel32[:], 0.0)
            nc.vector.memset(sel32[:, ds(0, 1)], s)

            halfones = consts.tile([1, 2 * NB], F16)
            nc.vector.memset(halfones[:], 0.5)

            w_sb = consts.tile([P, DCH * NN], F16)
            b_sb = consts.tile([1, NN], F32)
            # f16 twin of b so the rank-1 bias matmuls in the W-gated tail
            # chain run at 1 cyc/row instead of fp32's 4.
            b16_sb = consts.tile([1, NN], F16)

            for rep in range(reps):
                _emit_body(
                    nc, data, small, psum,
                    i_dram.ap(), j_dram.ap(), w_dram.ap(), b_dram.ap(),
                    o_view, sel, sel32, halfones, w_sb, b_sb, b16_sb,
                    load_wb=(rep == 0),
                )

    nc.compile()
    return nc


def _emit_body(nc, data, small, psum, i_ap, j_ap, w_ap, b_ap, o_view,
               sel, sel32, halfones, w_sb, b_sb, b16_sb, load_wb=True):
    # PSUM layout (fp32 cols per partition): uT's four d-chunk accumulation
    # chains at bank stride (cols cd*512), then one bank holding the dense
    # t = (v+b)/2 and m = (b-v)/2 results interleaved per n-chunk.
    ut_psum = psum.tile([P, DCH * PSB], F32)
    tm_psum = psum.tile([P, PSB], F32)

    # Bridge tile: the first 128 rows of batch 0's i ride a plain f32
    # HWDGE DMA.  The transfer occupies the DMA engines only during the
    # window where the first SWDGE (Pool) descriptor generation is still
    # running, so it comes to us almost free and shortens the casting
    # stream by one row-group.
    t32 = data.tile([P, D], F32, tag="t32")
    nc.sync.dma_start(out=t32[:], in_=i_ap[ds(0, P), :])

    if load_wb:
        # b is tiny and only feeds the rank-1 bias matmuls of the tail.
        nc.sync.dma_start(out=b_sb[:], in_=b_ap[:])
        nc.vector.tensor_copy(b16_sb[:], b_sb[:])

    # --- phase 1: uT[d, b] = (sum_l i[b,:,d] - sum_l j[b,:,d]) / 2L --------
    # Casting Pool DMAs halve the HBM->SBUF stream (the roofline term);
    # each [128, 128] f16 data chunk is then consumed as a matmul
    # *stationary* with the tiny selector column moving, so the whole
    # reduction costs the tensor engine almost nothing and produces uT in
    # the layout the dense tail wants.  Tiles pack consecutive DRAM rows
    # per partition line, so a whole batch element is one 128-descriptor
    # DMA.  The reduction only needs every row summed once -- which rows a
    # partition holds is irrelevant -- so the row->partition mapping is
    # free to chase descriptor shape.
    #
    # The bridge covers (i, batch 0, rows 0..127): open the four cd chains
    # with its f32 matmuls.
    for cd in range(DCH):
        nc.tensor.matmul(
            ut_psum[:, ds(cd * PSB, NB)],
            t32[:, ds(cd * P, P)],
            sel32[:],
            start=True,
            stop=False,
        )
    n_tiles = 2 * NB
    t_idx = 0
    for b in range(NB):
        for x_ap, blk in ((i_ap, b), (j_ap, NB + b)):
            if t_idx == 0:
                row0, rows = b * L + P, L - P  # bridge took rows 0..127
            else:
                row0, rows = b * L, L
            rp = rows // P  # consecutive rows per partition line
            th = data.tile([P, rp * D], F16, tag="t")
            nc.gpsimd.dma_start(
                out=th[:].rearrange("p (r n) -> p r n", r=rp),
                in_=x_ap[ds(row0, rows), :].rearrange(
                    "(p r) n -> p r n", r=rp
                ),
            )
            for r in range(rp):
                for cd in range(DCH):
                    nc.tensor.matmul(
                        ut_psum[:, ds(cd * PSB, NB)],
                        th[:, ds(r * D + cd * P, P)],
                        sel[:, ds(blk * NB, NB)],
                        start=False,
                        stop=(t_idx == n_tiles - 1 and r == rp - 1),
                    )
            t_idx += 1

    # W is only consumed by the dense tail, so its (casting) DMA queues
    # after the data stream: the last data tile lands ~1.5us earlier and W
    # streams in while the uT copy below runs.
    if load_wb:
        nc.gpsimd.dma_start(
            out=w_sb[:].rearrange("p (c n) -> p c n", c=DCH),
            in_=w_ap.rearrange("(c p) n -> p c n", p=P),
        )

    # --- phase 2: t[n,b] = (v+b)/2, m[n,b] = (b-v)/2, v = sum_d W[d,n] u[b,d]
    # +uT and -uT sit side by side in one [128, 8]-moving tile, so t and m
    # come from a single matmul chain per n-chunk (the stationary W block
    # is shared) and land interleaved in one PSUM bank.  The copies run
    # right after the last data tile and hide inside the W DMA + sem
    # window; the bias enters each chain as a rank-1 (K=1) matmul with a
    # 0.5-valued rhs.
    ut_view = ut_psum[:].rearrange("p (c x) -> p c x", x=PSB)[:, :, ds(0, NB)]
    ut_pm = small.tile([P, DCH * 2 * NB], F16)
    pm_view = ut_pm[:].rearrange("p (c s x) -> p c s x", s=2, x=NB)
    nc.vector.tensor_copy(pm_view[:, :, 0, :], ut_view)
    nc.vector.tensor_scalar_mul(pm_view[:, :, 1, :], ut_view, -1.0)
    for cn in range(NCH):
        for cd in range(DCH):
            nc.tensor.matmul(
                tm_psum[:, ds(cn * 2 * NB, 2 * NB)],
                w_sb[:, ds(cd * NN + cn * P, P)],
                ut_pm[:, ds(cd * 2 * NB, 2 * NB)],
                start=(cd == 0),
                stop=False,
            )
        nc.tensor.matmul(
            tm_psum[:, ds(cn * 2 * NB, 2 * NB)],
            b16_sb[:, ds(cn * P, P)],
            halfones[:],
            start=False,
            stop=True,
        )

    # --- phase 3: out = relu(t) + relu(m) ----------------------------------
    r_tm = small.tile([P, NCH * 2 * NB], F32)
    nc.vector.tensor_scalar_max(r_tm[:], tm_psum[:, ds(0, NCH * 2 * NB)], 0.0)
    r_view = r_tm[:].rearrange("p (c s x) -> p c s x", s=2, x=NB)
    o_sb = small.tile([P, NCH * NB], F32)
    nc.vector.tensor_add(
        o_sb[:].rearrange("p (c x) -> p c x", x=NB),
        r_view[:, :, 0, :],
        r_view[:, :, 1, :],
    )
    nc.sync.dma_start(out=o_view, in_=o_sb[:])


def _get_bass():
    if "nc" not in _CACHE:
        _CACHE["nc"] = _build_bass()
    return _CACHE["nc"]


def _make_in_maps(inputs):
    i = np.ascontiguousarray(np.asarray(inputs["i"], dtype=np.float32))
    j = np.ascontiguousarray(np.asarray(inputs["j"], dtype=np.float32))
    w = np.ascontiguousarray(np.asarray(inputs["W_agg"], dtype=np.float32))
    b = np.ascontiguousarray(
        np.asarray(inputs["b_agg"], dtype=np.float32).reshape(1, NN)
    )
    in_maps = []
    for c in range(NCORES):
        in_maps.append(
            {
                "i": i[c * NB : (c + 1) * NB].reshape(NB * L, D),
                "j": j[c * NB : (c + 1) * NB].reshape(NB * L, D),
                "w": w,
                "b": b,
            }
        )
    return in_maps


def run_traced(trace=False, **inputs):
    nc = _get_bass()
    in_maps = _make_in_maps(inputs)
    res = run_bass_kernel_spmd(nc, in_maps, list(range(NCORES)), trace=trace)
    # Device buffer is o_sb's native layout: out_sb[p, cn*NB + b] holds
    # output feature n = cn*128 + p of (core-local) batch b.
    out = np.concatenate(
        [
            res.results[c]["out"]
            .reshape(P, NCH, NB)
            .transpose(2, 1, 0)
            .reshape(NB, NN)
            for c in range(NCORES)
        ],
        axis=0,
    ).astype(np.float32)
    return out, res


def kernel(**inputs):
    out, _ = run_traced(trace=False, **inputs)
    return out
